# revision 40
# baseline (speedup 1.0000x reference)
"""Trainium2 Bass kernel: series decomposition (trend / seasonal / residual).

Reference, per sample (shape [C, H, W], all reductions along C):
    T = moving_average_k(x)   (window k, replicate-padded)  -> T = (MA @ x) / k
    S = phase_mean_pe(x - T)                                -> S = (W2 @ x) / (k*G)
    R = x - T - S
MA (band + edge clamp counts) and MP (phase indicator) are small nonnegative
integer matrices, G = C // pe. The detrend folds into the seasonal weights:
W2 = k*MP - MP@MA, still integer with |entries| <= ~91, so both reductions are
exact-integer fp16 matmuls applied directly to x.

Mapping: pure data parallel over batch across 8 NeuronCores. Channels live on
partitions (3 chunks of 112); both channel reductions run on TensorE. x is
split into fp16 hi + lo parts (x = hi + lo to ~2^-22 relative); both parts
accumulate into the same fp32 PSUM group -> fp32-class accuracy at full PE
speed. Integer weights are exact in fp16; the 1/k and 1/(k*G) scales ride on
the ScalarE PSUM->SBUF evacuation copies. R = x - T - S is computed by DVE
from the staged outputs. Weight blocks are zero-padded to M=128 output
columns (full PSUM tile + fp16 fast-weight-load).
"""

import numpy as np

N_CORES = 8
CK = 112   # channels per partition chunk
MPAD = 128  # weight blocks padded to 128 output columns (FWL + full PSUM tile)

_cache = {}


def _int_matrices(C, k, pe):
    """Integer numerators: MA is the trend band matrix (trend = MA@x / k);
    Ip [pe, C] is the 0/1 phase indicator; V = k*Ip - Ip@MA is the fused
    per-phase seasonal aggregate (W2 = Ip.T @ V gives S = W2@x / (k*G))."""
    p = (k - 1) // 2
    MA = np.zeros((C, C), dtype=np.int64)
    for m in range(C):
        for j in range(m - p, m + p + 1):
            MA[m, min(max(j, 0), C - 1)] += 1
    ph = np.arange(C) % pe
    Ip = (np.arange(pe)[:, None] == ph[None, :]).astype(np.int64)  # [pe, C]
    V = k * Ip - Ip @ MA
    return MA, V, Ip


def _pad_weights_T(W, NCH):
    """W [C, C] int -> transposed, per-output-chunk zero-padded [C, NCH*MPAD] f16."""
    C = W.shape[0]
    out = np.zeros((C, NCH * MPAD), dtype=np.float16)
    for mc in range(NCH):
        blk = W[mc * CK:(mc + 1) * CK, :].T  # [C(k), CK(m)]
        assert np.abs(blk).max() <= 2048
        out[:, mc * MPAD:mc * MPAD + CK] = blk.astype(np.float16)
    return out


def _build(BS, C, HW, k, pe, repeat=1, dma_only=False):
    """Build + compile the per-core Bass program.

    Per-core tensors: x [BS, C, HW] f32 in; wa/ws [C, NCH*MPAD] f16 in
    (transposed, block-padded integer weights); t/s/r [BS, C, HW] f32 out.
    `repeat` wraps the body in a hardware loop (timing variants only)."""
    import concourse.tile as tile
    from concourse import bacc, mybir

    f32, f16 = mybir.dt.float32, mybir.dt.float16
    NH = HW // 2  # one fp32 PSUM bank of matmul output columns
    NCH = C // CK
    assert C == NCH * CK and HW == 2 * NH

    MA, V, Ip = _int_matrices(C, k, pe)
    W2 = k * Ip.T @ Ip - (Ip.T @ Ip) @ MA  # fused seasonal: S = (W2 @ x)/(k*G)
    G = C // pe
    nzA = [
        [kc for kc in range(NCH) if MA[mc * CK:(mc + 1) * CK, kc * CK:(kc + 1) * CK].any()]
        for mc in range(NCH)
    ]
    nzS = [list(range(NCH))] * NCH

    nc = bacc.Bacc("TRN2", target_bir_lowering=False, debug=False, num_devices=N_CORES)
    x_d = nc.dram_tensor("x", [BS, C, HW], f32, kind="ExternalInput")
    wa_d = nc.dram_tensor("wa", [C, NCH * MPAD], f16, kind="ExternalInput")
    ws_d = nc.dram_tensor("ws", [C, NCH * MPAD], f16, kind="ExternalInput")
    t_d = nc.dram_tensor("t", [BS, C, HW], f32, kind="ExternalOutput")
    s_d = nc.dram_tensor("s", [BS, C, HW], f32, kind="ExternalOutput")
    r_d = nc.dram_tensor("r", [BS, C, HW], f32, kind="ExternalOutput")

    def chunked(ap2d):
        # [C, width] dram view -> [112 partitions, chunk, width]
        return ap2d.rearrange("(a p) n -> p a n", p=CK)

    with tile.TileContext(nc) as tc:
        with (
            tc.tile_pool(name="wpool", bufs=1) as wpool,
            tc.tile_pool(name="xpool", bufs=2) as xpool,
            tc.tile_pool(name="opool", bufs=2) as opool,
            tc.tile_pool(name="pspool", bufs=4, space="PSUM") as pspool,
        ):
            wtiles = {}

            def load_wa():
                wa = wpool.tile([CK, NCH, NCH * MPAD], f16, tag="wa")
                nc.sync.dma_start(out=wa[:], in_=chunked(wa_d.ap()))
                wtiles["wa"] = wa

            def load_ws():
                ws = wpool.tile([CK, NCH, NCH * MPAD], f16, tag="ws")
                nc.sync.dma_start(out=ws[:], in_=chunked(ws_d.ap()))
                wtiles["ws"] = ws

            def load_weights():
                load_wa()
                load_ws()

            def reduce_chunk(w, nz, xh, xl, scale, out_tile, mc, dram_row=None):
                """out[:112, mc] = (W @ (xh + xl))[mc chunk] * scale, fp32 PSUM groups.
                Evacuation + optional output DMA run per n-half for latency."""
                ps = pspool.tile([MPAD, 2, NH], f32, tag="ps")
                for h in range(2):
                    ops = [(kc, xp) for kc in nz[mc] for xp in (xh, xl)]
                    for j, (kc, xp) in enumerate(ops):
                        nc.tensor.matmul(
                            ps[:, h, :],
                            w[:, kc, mc * MPAD:(mc + 1) * MPAD],
                            xp[:, kc, h * NH:(h + 1) * NH],
                            start=(j == 0),
                            stop=(j == len(ops) - 1),
                        )
                # evacuate after BOTH bank groups complete (no ACT read of one
                # bank while PE writes the other bank of the same tile), still
                # per-half so each output DMA launches off a short evac
                for h in range(2):
                    hs = slice(h * NH, (h + 1) * NH)
                    nc.scalar.mul(out_tile[:, mc, hs], ps[:CK, h, :], scale)
                    if dram_row is not None:
                        nc.sync.dma_start(out=dram_row[:, hs], in_=out_tile[:, mc, hs])

            def load(b):
                xt = xpool.tile([CK, NCH, HW], f32, tag="xt", bufs=4)
                nc.sync.dma_start(out=xt[:], in_=chunked(x_d.ap()[b]))
                return xt

            def body_dma_only(xt, b, tiles):
                # Timing probe: same DMA traffic, no compute (outputs garbage).
                for mc in range(NCH):
                    row = slice(mc * CK, (mc + 1) * CK)
                    nc.sync.dma_start(out=t_d.ap()[b][row, :], in_=xt[:, mc, :])
                    nc.sync.dma_start(out=s_d.ap()[b][row, :], in_=xt[:, mc, :])
                    nc.sync.dma_start(out=r_d.ap()[b][row, :], in_=xt[:, mc, :])
                    if mc == 0 and b + 2 < BS:
                        tiles[b + 2] = load(b + 2)

            def split(xt):
                xh = xpool.tile([CK, NCH, HW], f16, tag="xh", bufs=3)
                nc.vector.tensor_copy(xh[:], xt[:])
                xl = xpool.tile([CK, NCH, HW], f16, tag="xl", bufs=3)
                nc.vector.tensor_sub(xl[:], xt[:], xh[:])
                return xh, xl

            def body(xt, b, tiles, splits):
                if dma_only:
                    return body_dma_only(xt, b, tiles)
                xh, xl = splits.pop(b)
                # hoist next sample's hi/lo split ahead of this sample's
                # residual ops so the in-order DVE queue never stalls PE
                if b + 1 < BS and (b + 1) in tiles:
                    splits[b + 1] = split(tiles[b + 1])

                tt = opool.tile([CK, NCH, HW], f32, tag="tt")
                st = opool.tile([CK, NCH, HW], f32, tag="st")
                rt = opool.tile([CK, NCH, HW], f32, tag="rt")
                last = b == BS - 1
                for mc in range(NCH):
                    row = slice(mc * CK, (mc + 1) * CK)
                    reduce_chunk(wtiles["wa"], nzA, xh, xl, 1.0 / k, tt, mc,
                                 dram_row=t_d.ap()[b][row, :])
                    reduce_chunk(wtiles["ws"], nzS, xh, xl, 1.0 / (k * G), st, mc,
                                 dram_row=s_d.ap()[b][row, :])
                    if last:
                        # half-granular residual: shortens the kernel tail
                        for h in range(2):
                            hs = slice(h * NH, (h + 1) * NH)
                            nc.vector.tensor_sub(rt[:, mc, hs], xt[:, mc, hs], tt[:, mc, hs])
                            nc.vector.tensor_sub(rt[:, mc, hs], rt[:, mc, hs], st[:, mc, hs])
                            nc.sync.dma_start(out=r_d.ap()[b][row, hs], in_=rt[:, mc, hs])
                    else:
                        nc.vector.tensor_sub(rt[:, mc, :], xt[:, mc, :], tt[:, mc, :])
                        nc.vector.tensor_sub(rt[:, mc, :], rt[:, mc, :], st[:, mc, :])
                        nc.sync.dma_start(out=r_d.ap()[b][row, :], in_=rt[:, mc, :])
                    if mc == 0 and b + 3 < BS:
                        # prefetch sample b+3 right behind the first output DMA
                        tiles[b + 3] = load(b + 3)

            def run_all(with_weights):
                # fast start: first sample arrives per k-chunk so the first
                # matmul group fires as soon as chunks 0+1 are split
                xt0 = xpool.tile([CK, NCH, HW], f32, tag="xt", bufs=4, name="xt0")
                xh0 = xpool.tile([CK, NCH, HW], f16, tag="xh", bufs=3, name="xh0")
                xl0 = xpool.tile([CK, NCH, HW], f16, tag="xl", bufs=3, name="xl0")
                for kc in range(NCH):
                    nc.sync.dma_start(
                        out=xt0[:, kc, :], in_=x_d.ap()[0][kc * CK:(kc + 1) * CK, :]
                    )
                    if kc == 0 and with_weights:
                        # trend weights right behind chunk 0; seasonal weights
                        # wait until all first-sample chunks are in flight
                        load_wa()
                    nc.vector.tensor_copy(xh0[:, kc, :], xt0[:, kc, :])
                    nc.vector.tensor_sub(xl0[:, kc, :], xt0[:, kc, :], xh0[:, kc, :])
                if with_weights:
                    load_ws()
                if not with_weights and "wa" not in wtiles:
                    load_weights()
                tiles = {0: xt0}
                for b in range(1, min(3, BS)):
                    tiles[b] = load(b)
                splits = {0: (xh0, xl0)}
                for b in range(BS):
                    body(tiles.pop(b), b, tiles, splits)

            if repeat == 1:
                run_all(with_weights=True)
            else:
                # weights are loop-invariant: load once outside the timing loop
                load_weights()
                ET = mybir.EngineType
                with tc.For_i(
                    0, repeat, 1,
                    hint_engines=(ET.PE, ET.DVE, ET.SP, ET.Activation, ET.Pool),
                ):
                    run_all(with_weights=False)

    nc.compile()
    return nc


class _SpmdExec:
    """Reusable executor: builds the sharded jit once, callable many times."""

    def __init__(self, nc, n_cores):
        import jax
        from jax.sharding import Mesh, PartitionSpec

        try:
            from jax.experimental.shard_map import shard_map
        except ImportError:
            from jax.shard_map import shard_map
        from concourse import bass2jax, mybir

        bass2jax.install_neuronx_cc_hook()
        self._jax = jax
        self.n_cores = n_cores
        partition_name = nc.partition_id_tensor.name if nc.partition_id_tensor else None
        in_names, out_names, out_avals = [], [], []
        for alloc in nc.m.functions[0].allocations:
            if not isinstance(alloc, mybir.MemoryLocationSet):
                continue
            name = alloc.memorylocations[0].name
            if alloc.kind == "ExternalInput":
                if name != partition_name:
                    in_names.append(name)
            elif alloc.kind == "ExternalOutput":
                out_names.append(name)
                out_avals.append(
                    jax.core.ShapedArray(tuple(alloc.tensor_shape), mybir.dt.np(alloc.dtype))
                )
        self.in_names, self.out_names, self.out_avals = in_names, out_names, out_avals
        all_in_names = list(in_names) + list(out_names)
        if partition_name is not None:
            all_in_names.append(partition_name)

        def _body(*args):
            operands = list(args)
            if partition_name is not None:
                operands.append(bass2jax.partition_id_tensor())
            outs = bass2jax._bass_exec_p.bind(
                *operands,
                out_avals=tuple(out_avals),
                in_names=tuple(all_in_names),
                out_names=tuple(out_names),
                lowering_input_output_aliases=(),
                sim_require_finite=True,
                sim_require_nnan=True,
                nc=nc,
            )
            return tuple(outs)

        devices = jax.devices()[:n_cores]
        mesh = Mesh(np.asarray(devices), ("core",))
        n_io = len(in_names) + len(out_avals)
        self._fn = jax.jit(
            shard_map(
                _body,
                mesh=mesh,
                in_specs=(PartitionSpec("core"),) * n_io,
                out_specs=(PartitionSpec("core"),) * len(out_avals),
                check_rep=False,
            ),
            keep_unused=True,
        )

    def prepare(self, in_maps):
        """Concat per-core inputs on axis 0 and put on device."""
        jax = self._jax
        concat = [
            np.concatenate([np.asarray(m[name]) for m in in_maps], axis=0)
            for name in self.in_names
        ]
        zeros = [
            np.zeros((self.n_cores * a.shape[0], *a.shape[1:]), a.dtype)
            for a in self.out_avals
        ]
        return [jax.device_put(a) for a in concat + zeros]

    def run(self, dev_args):
        out = self._fn(*dev_args)
        self._jax.block_until_ready(out)
        return out

    def __call__(self, in_maps):
        out = self.run(self.prepare(in_maps))
        res = []
        for c in range(self.n_cores):
            res.append(
                {
                    name: np.asarray(out[i]).reshape(
                        self.n_cores, *self.out_avals[i].shape
                    )[c]
                    for i, name in enumerate(self.out_names)
                }
            )
        return res


def _get_exec(BS, C, HW, k, pe, repeat=1, dma_only=False):
    key = (BS, C, HW, k, pe, repeat, dma_only)
    if key not in _cache:
        nc = _build(BS, C, HW, k, pe, repeat=repeat, dma_only=dma_only)
        _cache[key] = _SpmdExec(nc, N_CORES)
    return _cache[key]


def _in_maps(x3, C, k, pe):
    NCH = C // CK
    MA, V, Ip = _int_matrices(C, k, pe)
    W2 = k * Ip.T @ Ip - (Ip.T @ Ip) @ MA
    wa = _pad_weights_T(MA, NCH)
    ws = _pad_weights_T(W2, NCH)
    shards = np.split(x3, N_CORES, axis=0)
    return [{"x": np.ascontiguousarray(s), "wa": wa, "ws": ws} for s in shards]


def kernel(x, kernel_size, pe):
    x = np.asarray(x, dtype=np.float32)
    k = int(kernel_size)
    pe = int(pe)
    B, C, H, W = x.shape
    assert B % N_CORES == 0 and C % CK == 0
    BS = B // N_CORES
    HW = H * W

    in_maps = _in_maps(x.reshape(B, C, HW), C, k, pe)
    try:
        ex = _get_exec(BS, C, HW, k, pe)
        results = ex(in_maps)
    except Exception:
        # One retry with a freshly built executor (transient device errors).
        _cache.pop((BS, C, HW, k, pe, 1, False), None)
        ex = _get_exec(BS, C, HW, k, pe)
        results = ex(in_maps)

    def gather(name):
        full = np.concatenate([r[name] for r in results], axis=0)
        return full.reshape(B, C, H, W)

    return gather("t"), gather("s"), gather("r")


# revision 42
# speedup vs baseline: 1.0540x; 1.0540x over previous
"""Trainium2 Bass kernel: series decomposition (trend / seasonal / residual).

Reference, per sample (shape [C, H, W], all reductions along C):
    T = moving_average_k(x)   (window k, replicate-padded)  -> T = (MA @ x) / k
    S = phase_mean_pe(x - T)                                -> S = (W2 @ x) / (k*G)
    R = x - T - S
MA (band + edge clamp counts) and MP (phase indicator) are small nonnegative
integer matrices, G = C // pe. The detrend folds into the seasonal weights:
W2 = k*MP - MP@MA, still integer with |entries| <= ~91, so both reductions are
exact-integer fp16 matmuls applied directly to x.

Mapping: pure data parallel over batch across 8 NeuronCores. Channels live on
partitions (3 chunks of 112); both channel reductions run on TensorE. x is
split into fp16 hi + lo parts (x = hi + lo to ~2^-22 relative); both parts
accumulate into the same fp32 PSUM group -> fp32-class accuracy at full PE
speed. Integer weights are exact in fp16; the 1/k and 1/(k*G) scales ride on
the ScalarE PSUM->SBUF evacuation copies. R = x - T - S is computed by DVE
from the staged outputs. Weight blocks are zero-padded to M=128 output
columns (full PSUM tile + fp16 fast-weight-load).
"""

import numpy as np

N_CORES = 8
CK = 112   # channels per partition chunk
MPAD = 128  # weight blocks padded to 128 output columns (FWL + full PSUM tile)

_cache = {}


def _int_matrices(C, k, pe):
    """Integer numerators: MA is the trend band matrix (trend = MA@x / k);
    Ip [pe, C] is the 0/1 phase indicator; V = k*Ip - Ip@MA is the fused
    per-phase seasonal aggregate (W2 = Ip.T @ V gives S = W2@x / (k*G))."""
    p = (k - 1) // 2
    MA = np.zeros((C, C), dtype=np.int64)
    for m in range(C):
        for j in range(m - p, m + p + 1):
            MA[m, min(max(j, 0), C - 1)] += 1
    ph = np.arange(C) % pe
    Ip = (np.arange(pe)[:, None] == ph[None, :]).astype(np.int64)  # [pe, C]
    V = k * Ip - Ip @ MA
    return MA, V, Ip


def _pad_weights_T(W, NCH):
    """W [C, C] int -> transposed, per-output-chunk zero-padded [C, NCH*MPAD] f16."""
    C = W.shape[0]
    out = np.zeros((C, NCH * MPAD), dtype=np.float16)
    for mc in range(NCH):
        blk = W[mc * CK:(mc + 1) * CK, :].T  # [C(k), CK(m)]
        assert np.abs(blk).max() <= 2048
        out[:, mc * MPAD:mc * MPAD + CK] = blk.astype(np.float16)
    return out


def _build(BS, C, HW, k, pe, repeat=1, dma_only=False):
    """Build + compile the per-core Bass program.

    Per-core tensors: x [BS, C, HW] f32 in; wa/ws [C, NCH*MPAD] f16 in
    (transposed, block-padded integer weights); t/s/r [BS, C, HW] f32 out.
    `repeat` wraps the body in a hardware loop (timing variants only)."""
    import concourse.tile as tile
    from concourse import bacc, mybir

    f32, f16 = mybir.dt.float32, mybir.dt.float16
    NH = HW // 2  # one fp32 PSUM bank of matmul output columns
    NCH = C // CK
    assert C == NCH * CK and HW == 2 * NH

    MA, V, Ip = _int_matrices(C, k, pe)
    W2 = k * Ip.T @ Ip - (Ip.T @ Ip) @ MA  # fused seasonal: S = (W2 @ x)/(k*G)
    G = C // pe
    nzA = [
        [kc for kc in range(NCH) if MA[mc * CK:(mc + 1) * CK, kc * CK:(kc + 1) * CK].any()]
        for mc in range(NCH)
    ]
    nzS = [list(range(NCH))] * NCH

    nc = bacc.Bacc("TRN2", target_bir_lowering=False, debug=False, num_devices=N_CORES)
    x_d = nc.dram_tensor("x", [BS, C, HW], f32, kind="ExternalInput")
    wa_d = nc.dram_tensor("wa", [C, NCH * MPAD], f16, kind="ExternalInput")
    ws_d = nc.dram_tensor("ws", [C, NCH * MPAD], f16, kind="ExternalInput")
    t_d = nc.dram_tensor("t", [BS, C, HW], f32, kind="ExternalOutput")
    s_d = nc.dram_tensor("s", [BS, C, HW], f32, kind="ExternalOutput")
    r_d = nc.dram_tensor("r", [BS, C, HW], f32, kind="ExternalOutput")

    def chunked(ap2d):
        # [C, width] dram view -> [112 partitions, chunk, width]
        return ap2d.rearrange("(a p) n -> p a n", p=CK)

    with tile.TileContext(nc) as tc:
        with (
            tc.tile_pool(name="wpool", bufs=1) as wpool,
            tc.tile_pool(name="xpool", bufs=2) as xpool,
            tc.tile_pool(name="opool", bufs=2) as opool,
            tc.tile_pool(name="pspool", bufs=4, space="PSUM") as pspool,
        ):
            wtiles = {}

            def load_wa():
                wa = wpool.tile([CK, NCH, NCH * MPAD], f16, tag="wa")
                nc.sync.dma_start(out=wa[:], in_=chunked(wa_d.ap()))
                wtiles["wa"] = wa

            def load_ws():
                ws = wpool.tile([CK, NCH, NCH * MPAD], f16, tag="ws")
                nc.sync.dma_start(out=ws[:], in_=chunked(ws_d.ap()))
                wtiles["ws"] = ws

            def load_weights():
                load_wa()
                load_ws()

            def reduce_chunk(w, nz, xh, xl, scale, out_tile, mc, dram_row=None):
                """out[:112, mc] = (W @ (xh + xl))[mc chunk] * scale, fp32 PSUM groups.
                Evacuation + optional output DMA run per n-half for latency."""
                ps = pspool.tile([MPAD, 2, NH], f32, tag="ps")
                for h in range(2):
                    ops = [(kc, xp) for kc in nz[mc] for xp in (xh, xl)]
                    for j, (kc, xp) in enumerate(ops):
                        nc.tensor.matmul(
                            ps[:, h, :],
                            w[:, kc, mc * MPAD:(mc + 1) * MPAD],
                            xp[:, kc, h * NH:(h + 1) * NH],
                            start=(j == 0),
                            stop=(j == len(ops) - 1),
                        )
                # evacuate after BOTH bank groups complete (no ACT read of one
                # bank while PE writes the other bank of the same tile), still
                # per-half so each output DMA launches off a short evac
                for h in range(2):
                    hs = slice(h * NH, (h + 1) * NH)
                    nc.scalar.mul(out_tile[:, mc, hs], ps[:CK, h, :], scale)
                    if dram_row is not None:
                        nc.sync.dma_start(out=dram_row[:, hs], in_=out_tile[:, mc, hs])

            def load(b):
                xt = xpool.tile([CK, NCH, HW], f32, tag="xt", bufs=4)
                nc.sync.dma_start(out=xt[:], in_=chunked(x_d.ap()[b]))
                return xt

            def body_dma_only(xt, b, tiles):
                # Timing probe: same DMA traffic, no compute (outputs garbage).
                for mc in range(NCH):
                    row = slice(mc * CK, (mc + 1) * CK)
                    nc.sync.dma_start(out=t_d.ap()[b][row, :], in_=xt[:, mc, :])
                    nc.sync.dma_start(out=s_d.ap()[b][row, :], in_=xt[:, mc, :])
                    nc.sync.dma_start(out=r_d.ap()[b][row, :], in_=xt[:, mc, :])
                    if mc == 0 and b + 2 < BS:
                        tiles[b + 2] = load(b + 2)

            def split(xt):
                xh = xpool.tile([CK, NCH, HW], f16, tag="xh", bufs=3)
                nc.vector.tensor_copy(xh[:], xt[:])
                xl = xpool.tile([CK, NCH, HW], f16, tag="xl", bufs=3)
                nc.vector.tensor_sub(xl[:], xt[:], xh[:])
                return xh, xl

            def body(xt, b, tiles, splits):
                if dma_only:
                    return body_dma_only(xt, b, tiles)
                xh, xl = splits.pop(b)
                # hoist next sample's hi/lo split ahead of this sample's
                # residual ops so the in-order DVE queue never stalls PE
                if b + 1 < BS and (b + 1) in tiles:
                    splits[b + 1] = split(tiles[b + 1])

                tt = opool.tile([CK, NCH, HW], f32, tag="tt")
                st = opool.tile([CK, NCH, HW], f32, tag="st")
                rt = opool.tile([CK, NCH, HW], f32, tag="rt")
                last = b == BS - 1
                for mc in range(NCH):
                    row = slice(mc * CK, (mc + 1) * CK)
                    reduce_chunk(wtiles["wa"], nzA, xh, xl, 1.0 / k, tt, mc,
                                 dram_row=t_d.ap()[b][row, :])
                    reduce_chunk(wtiles["ws"], nzS, xh, xl, 1.0 / (k * G), st, mc,
                                 dram_row=s_d.ap()[b][row, :])
                    if last:
                        # half-granular residual: shortens the kernel tail
                        for h in range(2):
                            hs = slice(h * NH, (h + 1) * NH)
                            nc.vector.tensor_sub(rt[:, mc, hs], xt[:, mc, hs], tt[:, mc, hs])
                            nc.vector.tensor_sub(rt[:, mc, hs], rt[:, mc, hs], st[:, mc, hs])
                            nc.sync.dma_start(out=r_d.ap()[b][row, hs], in_=rt[:, mc, hs])
                    else:
                        nc.vector.tensor_sub(rt[:, mc, :], xt[:, mc, :], tt[:, mc, :])
                        nc.vector.tensor_sub(rt[:, mc, :], rt[:, mc, :], st[:, mc, :])
                        nc.sync.dma_start(out=r_d.ap()[b][row, :], in_=rt[:, mc, :])
                    if mc == 0 and b + 3 < BS:
                        # prefetch sample b+3 right behind the first output DMA
                        tiles[b + 3] = load(b + 3)

            def run_all(with_weights):
                # fast start: first sample arrives per k-chunk so the first
                # matmul group fires as soon as chunks 0+1 are split
                xt0 = xpool.tile([CK, NCH, HW], f32, tag="xt", bufs=4, name="xt0")
                xh0 = xpool.tile([CK, NCH, HW], f16, tag="xh", bufs=3, name="xh0")
                xl0 = xpool.tile([CK, NCH, HW], f16, tag="xl", bufs=3, name="xl0")
                for kc in range(NCH):
                    nc.sync.dma_start(
                        out=xt0[:, kc, :], in_=x_d.ap()[0][kc * CK:(kc + 1) * CK, :]
                    )
                    if kc == 0 and with_weights:
                        # trend weights right behind chunk 0; seasonal weights
                        # wait until all first-sample chunks are in flight
                        load_wa()
                    nc.vector.tensor_copy(xh0[:, kc, :], xt0[:, kc, :])
                    nc.vector.tensor_sub(xl0[:, kc, :], xt0[:, kc, :], xh0[:, kc, :])
                if with_weights:
                    load_ws()
                if not with_weights and "wa" not in wtiles:
                    load_weights()
                tiles = {0: xt0}
                for b in range(1, min(3, BS)):
                    tiles[b] = load(b)
                splits = {0: (xh0, xl0)}
                for b in range(BS):
                    body(tiles.pop(b), b, tiles, splits)

            if repeat == 1:
                run_all(with_weights=True)
            else:
                # weights are loop-invariant: load once outside the timing loop
                load_weights()
                ET = mybir.EngineType
                with tc.For_i(
                    0, repeat, 1,
                    hint_engines=(ET.PE, ET.DVE, ET.SP, ET.Activation, ET.Pool),
                ):
                    run_all(with_weights=False)

    nc.compile()
    return nc


class _SpmdExec:
    """Reusable executor: builds the sharded jit once, callable many times."""

    def __init__(self, nc, n_cores):
        import jax
        from jax.sharding import Mesh, PartitionSpec

        try:
            from jax.experimental.shard_map import shard_map
        except ImportError:
            from jax.shard_map import shard_map
        from concourse import bass2jax, mybir

        bass2jax.install_neuronx_cc_hook()
        self._jax = jax
        self.n_cores = n_cores
        partition_name = nc.partition_id_tensor.name if nc.partition_id_tensor else None
        in_names, out_names, out_avals = [], [], []
        for alloc in nc.m.functions[0].allocations:
            if not isinstance(alloc, mybir.MemoryLocationSet):
                continue
            name = alloc.memorylocations[0].name
            if alloc.kind == "ExternalInput":
                if name != partition_name:
                    in_names.append(name)
            elif alloc.kind == "ExternalOutput":
                out_names.append(name)
                out_avals.append(
                    jax.core.ShapedArray(tuple(alloc.tensor_shape), mybir.dt.np(alloc.dtype))
                )
        self.in_names, self.out_names, self.out_avals = in_names, out_names, out_avals
        all_in_names = list(in_names) + list(out_names)
        if partition_name is not None:
            all_in_names.append(partition_name)

        def _body(*args):
            operands = list(args)
            if partition_name is not None:
                operands.append(bass2jax.partition_id_tensor())
            outs = bass2jax._bass_exec_p.bind(
                *operands,
                out_avals=tuple(out_avals),
                in_names=tuple(all_in_names),
                out_names=tuple(out_names),
                lowering_input_output_aliases=(),
                sim_require_finite=True,
                sim_require_nnan=True,
                nc=nc,
            )
            return tuple(outs)

        devices = jax.devices()[:n_cores]
        mesh = Mesh(np.asarray(devices), ("core",))
        n_io = len(in_names) + len(out_avals)
        self._fn = jax.jit(
            shard_map(
                _body,
                mesh=mesh,
                in_specs=(PartitionSpec("core"),) * n_io,
                out_specs=(PartitionSpec("core"),) * len(out_avals),
                check_rep=False,
            ),
            keep_unused=True,
        )

    def prepare(self, in_maps):
        """Concat per-core inputs on axis 0 and put on device."""
        jax = self._jax
        concat = [
            np.concatenate([np.asarray(m[name]) for m in in_maps], axis=0)
            for name in self.in_names
        ]
        zeros = [
            np.zeros((self.n_cores * a.shape[0], *a.shape[1:]), a.dtype)
            for a in self.out_avals
        ]
        return [jax.device_put(a) for a in concat + zeros]

    def run(self, dev_args):
        out = self._fn(*dev_args)
        self._jax.block_until_ready(out)
        return out

    def __call__(self, in_maps):
        out = self.run(self.prepare(in_maps))
        res = []
        for c in range(self.n_cores):
            res.append(
                {
                    name: np.asarray(out[i]).reshape(
                        self.n_cores, *self.out_avals[i].shape
                    )[c]
                    for i, name in enumerate(self.out_names)
                }
            )
        return res


def _get_exec(BS, C, HW, k, pe, repeat=1, dma_only=False):
    key = (BS, C, HW, k, pe, repeat, dma_only)
    if key not in _cache:
        nc = _build(BS, C, HW, k, pe, repeat=repeat, dma_only=dma_only)
        _cache[key] = _SpmdExec(nc, N_CORES)
    return _cache[key]


def _in_maps(x3, C, k, pe):
    NCH = C // CK
    MA, V, Ip = _int_matrices(C, k, pe)
    W2 = k * Ip.T @ Ip - (Ip.T @ Ip) @ MA
    wa = _pad_weights_T(MA, NCH)
    ws = _pad_weights_T(W2, NCH)
    shards = np.split(x3, N_CORES, axis=0)
    return [{"x": np.ascontiguousarray(s), "wa": wa, "ws": ws} for s in shards]


def kernel(x, kernel_size, pe):
    x = np.asarray(x, dtype=np.float32)
    k = int(kernel_size)
    pe = int(pe)
    B, C, H, W = x.shape
    assert B % N_CORES == 0 and C % CK == 0
    BS = B // N_CORES
    HW = H * W

    in_maps = _in_maps(x.reshape(B, C, HW), C, k, pe)
    try:
        ex = _get_exec(BS, C, HW, k, pe)
        results = ex(in_maps)
    except Exception:
        # One retry with a freshly built executor (transient device errors).
        _cache.pop((BS, C, HW, k, pe, 1, False), None)
        ex = _get_exec(BS, C, HW, k, pe)
        results = ex(in_maps)

    def gather(name):
        full = np.concatenate([r[name] for r in results], axis=0)
        return full.reshape(B, C, H, W)

    return gather("t"), gather("s"), gather("r")
